# revision 21
# baseline (speedup 1.0000x reference)
"""Trainium2 Bass kernel: NeuralLogicMachine LogicLayer message passing.

out[n] = sum_t w_t * mean_{e: src_e=n} relu(x[src_e] @ A_t + x[dst_e] @ B_t + b_t)

Strategy (8 NeuronCores, SPMD, no collectives):
  - Shard NODES by src range: core c owns src in [c*NPC, (c+1)*NPC).
    Each core receives all edges whose src is in its range (all 5 types)
    and produces the final output rows for its range; host concatenates.
  - On device per core:
      Phase 1: P[t, n_local] = x[n] @ A_t + b_t          (PE matmuls)
      Phase 2: stream edges in windows:
        dma_gather   Pg = P[t, src_rel]          (512B rows)
        dma_gather   xd = x[dst]                 (512B rows)
        PE: transpose xd chunks; psum = xdT.T @ B_t; += Pg (fp32r identity mm)
        ACT: proj = relu(scale_e * psum),  scale_e = w_t / max(deg_t(src),1)
        dma_scatter_add out[src_rel] += proj
  - Scatter windows are built host-side with round-robin dealing so all
    scatter indices inside one window are unique (no DMA RMW races);
    successive windows are serialized by Tile's WAW dependency.
"""

import sys
import numpy as np

for _p in ("/opt/trn_rl_repo", "/opt/pypackages"):
    if _p not in sys.path:
        sys.path.insert(0, _p)

from concourse import bacc, mybir  # noqa: E402
import concourse.tile as tile  # noqa: E402
from concourse.masks import make_identity  # noqa: E402

F32 = mybir.dt.float32
F32R = mybir.dt.float32r
I16 = mybir.dt.int16


def full_cfg():
    return dict(
        N=50000, C=128, T=5, E=200000, NCORES=8,
        NPC=6250,          # nodes per core (src shard)
        PROWS=6272,        # padded rows per type in P table & output (49*128)
        HALF=25000,        # x row split so gather idx fits int16
        WINDOW=1024,       # tokens per scatter window (8 chunks)
        GROUP_WINS=1,      # windows per gather call (num_idxs<=1024 HW limit)
    )


def small_cfg():
    return dict(
        N=2048, C=128, T=5, E=4096, NCORES=8,
        NPC=256, PROWS=384, HALF=1024, WINDOW=256, GROUP_WINS=2,
    )


def medium_cfg():
    return dict(
        N=8192, C=128, T=5, E=65536, NCORES=8,
        NPC=1024, PROWS=1152, HALF=4096, WINDOW=1024, GROUP_WINS=1,
    )


def _softmax(v):
    v = np.asarray(v, np.float64)
    e = np.exp(v - v.max())
    return (e / e.sum()).astype(np.float32)


def build_schedule(cfg, edge_index, edge_attention):
    """Host-side: shard + bucket + deal edges into fixed-size windows.

    Returns (groups, n_tok, per_core):
      groups:   list of (t, h, n_win, tok_off); every window = WINDOW tokens
      n_tok:    tokens per core (uniform)
      per_core: dicts with pg_idx/xd_idx/sc_idx [128, n_tok//16] int16 and
                s_scale [128, n_tok//128] float32
    """
    N, T = cfg["N"], cfg["T"]
    NC, NPC, PROWS, HALF, W = (
        cfg["NCORES"], cfg["NPC"], cfg["PROWS"], cfg["HALF"], cfg["WINDOW"])
    src = np.asarray(edge_index[:, 0, :], np.int64)  # [T,E]
    dst = np.asarray(edge_index[:, 1, :], np.int64)
    w_soft = _softmax(edge_attention)

    deg = np.stack([np.bincount(src[t], minlength=N) for t in range(T)])
    scale_tbl = w_soft[:, None] / np.maximum(deg, 1).astype(np.float32)

    core_of = src // NPC
    half_of = (dst >= HALF).astype(np.int64)

    # window counts per (t,h): uniform across cores, >= max src multiplicity
    n_win = np.zeros((T, 2), np.int64)
    buckets = {}  # (c,t,h) -> edge ids sorted by src
    for t in range(T):
        for h in range(2):
            max_cnt, max_mult = 0, 0
            for c in range(NC):
                ids = np.nonzero((core_of[t] == c) & (half_of[t] == h))[0]
                if ids.size:
                    ids = ids[np.argsort(src[t][ids], kind="stable")]
                    _, cnts = np.unique(src[t][ids], return_counts=True)
                    max_mult = max(max_mult, int(cnts.max()))
                max_cnt = max(max_cnt, ids.size)
                buckets[(c, t, h)] = ids
            n_win[t, h] = max(1, -(-max_cnt // W), max_mult)

    GW = cfg["GROUP_WINS"]
    per_bucket, tok_off, win_off = {}, 0, {}
    for t in range(T):
        for h in range(2):
            win_off[(t, h)] = tok_off
            gl = []
            left = int(n_win[t, h])
            while left > 0:
                g = min(GW, left)
                gl.append((t, h, g, tok_off))
                tok_off += g * W
                left -= g
            per_bucket[(t, h)] = gl
    # round-robin across buckets so the pool engine always has an
    # independent window to prefetch while one window's scatter waits
    groups = []
    lists = [list(v) for v in per_bucket.values()]
    while any(lists):
        for gl in lists:
            if gl:
                groups.append(gl.pop(0))
    n_tok = tok_off
    assert n_tok % 128 == 0
    assert T * PROWS < 32768 and PROWS < 32768 and (N - HALF) < 32768

    per_core = []
    for c in range(NC):
        base = c * NPC
        pg = np.zeros(n_tok, np.int64)
        xd = np.zeros(n_tok, np.int64)
        sc = np.full(n_tok, NPC, np.int64)   # dummies -> trash row
        sv = np.zeros(n_tok, np.float32)
        for t in range(T):
            pg_base = t * PROWS
            for h in range(2):
                ids = buckets[(c, t, h)]
                nw = int(n_win[t, h])
                off0 = win_off[(t, h)]
                pg[off0:off0 + nw * W] = pg_base + NPC  # default: defined row
                for wdx in range(nw):
                    wids = ids[wdx::nw]
                    assert wids.size <= W, (c, t, h, wdx, wids.size)
                    o = off0 + wdx * W
                    sl = slice(o, o + wids.size)
                    s_rel = src[t][wids] - base
                    pg[sl] = pg_base + s_rel
                    xd[sl] = dst[t][wids] - HALF * h
                    sc[sl] = s_rel
                    sv[sl] = scale_tbl[t][src[t][wids]]

        def wrap16(vals):
            # token j -> [j % 16, j // 16]; the 16-partition block is
            # replicated across all 8 GPSIMD cores (partitions 16..127)
            blk = vals.reshape(n_tok // 16, 16).T
            return np.ascontiguousarray(np.tile(blk, (8, 1)).astype(np.int16))

        per_core.append(dict(
            pg_idx=wrap16(pg), xd_idx=wrap16(xd), sc_idx=wrap16(sc),
            s_scale=np.ascontiguousarray(
                sv.reshape(n_tok // 128, 128).T).astype(np.float32),
        ))
    return groups, n_tok, per_core


def build_bass(cfg, groups, n_tok):
    """Emit the Bass/Tile program (identical for all cores)."""
    C, T = cfg["C"], cfg["T"]
    NPC, PROWS, HALF, W = cfg["NPC"], cfg["PROWS"], cfg["HALF"], cfg["WINDOW"]
    XPAD = cfg["N"] - NPC + PROWS          # padded x rows
    PT = PROWS // 128                      # P tiles per type
    CW = W // 128                          # chunks per window
    QB = min(512, W)                       # tokens per PE/ACT sub-batch
    KQ = QB // 128                         # chunks per sub-batch
    n_chunks = n_tok // 128

    nc = bacc.Bacc("TRN2", target_bir_lowering=False, debug=False,
                   num_devices=cfg["NCORES"])

    x_d = nc.dram_tensor("x", [XPAD, C], F32, kind="ExternalInput").ap()
    xl_d = nc.dram_tensor("xloc", [PROWS, C], F32, kind="ExternalInput").ap()
    w_d = nc.dram_tensor("W", [T, 2 * C, C], F32, kind="ExternalInput").ap()
    b_d = nc.dram_tensor("b", [T, C], F32, kind="ExternalInput").ap()
    pgi_d = nc.dram_tensor("pg_idx", [128, n_tok // 16], I16,
                           kind="ExternalInput").ap()
    xdi_d = nc.dram_tensor("xd_idx", [128, n_tok // 16], I16,
                           kind="ExternalInput").ap()
    sci_d = nc.dram_tensor("sc_idx", [128, n_tok // 16], I16,
                           kind="ExternalInput").ap()
    s_d = nc.dram_tensor("s_scale", [128, n_chunks], F32,
                         kind="ExternalInput").ap()
    p_d = nc.dram_tensor("P", [T * PROWS, C], F32, kind="Internal").ap()
    out_d = nc.dram_tensor("out", [PROWS, C], F32, kind="ExternalOutput").ap()

    with tile.TileContext(nc) as tc:
        with (
            tc.tile_pool(name="const", bufs=1) as cpool,
            tc.tile_pool(name="xin", bufs=2) as xpool,
            tc.tile_pool(name="pout", bufs=2) as ppool,
            tc.tile_pool(name="grp", bufs=4) as gpool,
            tc.tile_pool(name="xdt", bufs=4) as tpool,
            tc.tile_pool(name="ps_pre", bufs=1, space="PSUM") as pre_ps,
            tc.tile_pool(name="ps_tr", bufs=3, space="PSUM") as tr_ps,
            tc.tile_pool(name="ps_mm", bufs=3, space="PSUM") as mm_ps,
        ):
            ident = cpool.tile([128, 128], F32)
            make_identity(nc, ident[:])
            ones = cpool.tile([1, 128], F32)
            nc.gpsimd.memset(ones[:], 1.0)
            zero = cpool.tile([128, PT * 128], F32)
            nc.gpsimd.memset(zero[:], 0.0)

            ab = cpool.tile([128, 2 * T * 128], F32)  # [A_0|B_0|A_1|B_1|...]
            for t in range(T):
                nc.sync.dma_start(ab[:, (2 * t) * 128:(2 * t + 1) * 128],
                                  w_d[t, 0:128, :])
                nc.sync.dma_start(ab[:, (2 * t + 1) * 128:(2 * t + 2) * 128],
                                  w_d[t, 128:256, :])
            bb = cpool.tile([1, T * 128], F32)
            for t in range(T):
                nc.sync.dma_start(bb[0:1, t * 128:(t + 1) * 128],
                                  b_d[t:t + 1, :])

            pgi = cpool.tile([128, n_tok // 16], I16)
            nc.sync.dma_start(pgi[:], pgi_d[:])
            xdi = cpool.tile([128, n_tok // 16], I16)
            nc.sync.dma_start(xdi[:], xdi_d[:])
            sci = cpool.tile([128, n_tok // 16], I16)
            nc.sync.dma_start(sci[:], sci_d[:])
            ssc = cpool.tile([128, n_chunks], F32)
            nc.sync.dma_start(ssc[:], s_d[:])

            # zero the output accumulator
            out_r = out_d.rearrange("(n p) f -> p n f", p=128)
            nc.sync.dma_start(out_r[:, :, :],
                              zero[:].rearrange("p (n f) -> p n f", f=128))

            # ---- Phase 1: P[t] = xloc @ A_t + ones x b_t ----
            for k in range(PT):
                xt_sb = xpool.tile([128, 128], F32, tag="xt")
                nc.sync.dma_start(xt_sb[:], xl_d[k * 128:(k + 1) * 128, :])
                ps_t = pre_ps.tile([128, 128], F32, tag="pxt")
                nc.tensor.transpose(ps_t[:], xt_sb[:], ident[:])
                xT = xpool.tile([128, 128], F32, tag="xT")
                nc.vector.tensor_copy(xT[:], ps_t[:])
                for t in range(T):
                    ps_p = pre_ps.tile([128, 128], F32, tag="pp")
                    nc.tensor.matmul(ps_p[:], xT[:],
                                     ab[:, (2 * t) * 128:(2 * t + 1) * 128],
                                     start=True, stop=False,
                                     skip_group_check=True)
                    nc.tensor.matmul(ps_p[:], ones[:],
                                     bb[0:1, t * 128:(t + 1) * 128],
                                     start=False, stop=True,
                                     skip_group_check=True)
                    p_sb = ppool.tile([128, 128], F32)
                    nc.vector.tensor_copy(p_sb[:], ps_p[:])
                    nc.sync.dma_start(
                        p_d[t * PROWS + k * 128:t * PROWS + (k + 1) * 128, :],
                        p_sb[:])

            # ---- Phase 2: edge stream ----
            relu = mybir.ActivationFunctionType.Relu
            for (t, h, nw, off) in groups:
                ntk = nw * W
                ch0 = off // 128
                pg = gpool.tile([128, ntk], F32, tag="pg")
                nc.gpsimd.dma_gather(
                    pg[:].rearrange("p (n e) -> p n e", e=128),
                    p_d[:, :], pgi[:, off // 16:(off + ntk) // 16],
                    ntk, ntk, 128)
                xg = gpool.tile([128, ntk], F32, tag="xg")
                reg = x_d[0:HALF, :] if h == 0 else x_d[HALF:XPAD, :]
                nc.gpsimd.dma_gather(
                    xg[:].rearrange("p (n e) -> p n e", e=128),
                    reg, xdi[:, off // 16:(off + ntk) // 16],
                    ntk, ntk, 128)
                proj = gpool.tile([128, ntk], F32, tag="proj")
                for wdx in range(nw):
                    for q in range(W // QB):
                        qb = wdx * W + q * QB  # token offset within group
                        ps_t = tr_ps.tile([128, QB], F32, tag="tr")
                        for k in range(KQ):
                            nc.tensor.transpose(
                                ps_t[:, k * 128:(k + 1) * 128],
                                xg[:, qb + k * 128:qb + (k + 1) * 128],
                                ident[:])
                        xdT = tpool.tile([128, QB], F32)
                        nc.vector.tensor_copy(xdT[:], ps_t[:])
                        ps_m = mm_ps.tile([128, QB], F32, tag="mm")
                        for k in range(KQ):
                            nc.tensor.matmul(
                                ps_m[:, k * 128:(k + 1) * 128],
                                xdT[:, k * 128:(k + 1) * 128],
                                ab[:, (2 * t + 1) * 128:(2 * t + 2) * 128],
                                start=True, stop=True, skip_group_check=True)
                        sm = tpool.tile([128, QB], F32, tag="sum")
                        nc.vector.tensor_add(sm[:], ps_m[:],
                                             pg[:, qb:qb + QB])
                        for k in range(KQ):
                            gc = ch0 + (qb + k * 128) // 128
                            nc.scalar.activation(
                                proj[:, qb + k * 128:qb + (k + 1) * 128],
                                sm[:, k * 128:(k + 1) * 128],
                                relu, scale=ssc[:, gc:gc + 1])
                    so = off + wdx * W
                    nc.gpsimd.dma_scatter_add(
                        out_d[:, :],
                        proj[:].rearrange("p (n e) -> p n e", e=128)[
                            :, wdx * CW:(wdx + 1) * CW, :],
                        sci[:, so // 16:(so + W) // 16], W, W, 128)

    nc.compile()
    return nc


def _in_maps(cfg, x, W, b, per_core):
    N, NPC, PROWS = cfg["N"], cfg["NPC"], cfg["PROWS"]
    XPAD = N - NPC + PROWS
    x = np.asarray(x, np.float32)
    x_pad = np.zeros((XPAD, cfg["C"]), np.float32)
    x_pad[:N] = x
    maps = []
    for c in range(cfg["NCORES"]):
        base = c * NPC
        maps.append(dict(
            x=x_pad,
            xloc=np.ascontiguousarray(x_pad[base:base + PROWS]),
            W=np.asarray(W, np.float32),
            b=np.asarray(b, np.float32),
            **per_core[c],
        ))
    return maps


def run(cfg, inputs, backend="hw", trace=False):
    """Build schedule + program, run on all cores, return [N, C] output."""
    groups, n_tok, per_core = build_schedule(
        cfg, np.asarray(inputs["edge_index"]),
        np.asarray(inputs["edge_attention"]))
    nc = build_bass(cfg, groups, n_tok)
    maps = _in_maps(cfg, inputs["x"], inputs["W"], inputs["b"], per_core)
    NPC = cfg["NPC"]

    if backend == "sim":
        import os
        from concourse.bass_interp import CoreSim
        outs = []
        for c in range(int(os.environ.get("SIM_CORES", cfg["NCORES"]))):
            sim = CoreSim(nc)
            for k, v in maps[c].items():
                sim.tensor(k)[:] = v
            if nc.partition_id_tensor is not None:
                sim.tensor(nc.partition_id_tensor.name)[:] = np.array(
                    [[c]], dtype=np.uint32)
            sim.simulate(check_with_hw=False)
            outs.append(np.array(sim.tensor("out"))[:NPC])
        return np.concatenate(outs, 0), None

    if trace:
        _install_ntff_hook()
    from concourse import bass_utils
    if trace:
        bass_utils.upload_artifacts = lambda tmpdir: "local://" + str(tmpdir)
    try:
        res = bass_utils.run_bass_kernel_spmd(
            nc, maps, core_ids=list(range(cfg["NCORES"])), trace=trace)
    except Exception:
        if not trace:
            raise
        import traceback
        traceback.print_exc()
        print("trace run failed; retrying without trace", file=sys.stderr)
        res = bass_utils.run_bass_kernel_spmd(
            nc, maps, core_ids=list(range(cfg["NCORES"])), trace=False)
    out = np.concatenate([res.results[c]["out"][:NPC]
                          for c in range(cfg["NCORES"])], 0)
    return out, res.exec_time_ns


def _install_ntff_hook():
    """Provide antenv.axon_hooks (absent from this image) and register the
    ctypes NTFF profiling hook from the axon boot helper."""
    import types
    if "antenv.axon_hooks" in sys.modules:
        return
    mod = types.ModuleType("antenv.axon_hooks")
    state = {"hook": None}
    mod.set_axon_ntff_profile_hook = lambda h: state.__setitem__("hook", h)
    mod.get_axon_ntff_profile_hook = lambda: state["hook"]
    import antenv
    sys.modules["antenv.axon_hooks"] = mod
    antenv.axon_hooks = mod
    if "/root/.axon_site" not in sys.path:
        sys.path.append("/root/.axon_site")
    try:
        from trn_agent_boot.trn_boot import _ntff_profile_via_ctypes
        mod.set_axon_ntff_profile_hook(
            _ntff_profile_via_ctypes("/opt/axon/libaxon_pjrt.so"))
    except Exception as e:  # hook stays None -> bass_utils skips tracing
        print("ntff hook unavailable:", e, file=sys.stderr)


def kernel(**inputs):
    out, _ = run(full_cfg(), inputs, backend="hw")
    return out.astype(np.float32)


# revision 23
# speedup vs baseline: 1.0046x; 1.0046x over previous
"""Trainium2 Bass kernel: NeuralLogicMachine LogicLayer message passing.

out[n] = sum_t w_t * mean_{e: src_e=n} relu(x[src_e] @ A_t + x[dst_e] @ B_t + b_t)

Strategy (8 NeuronCores, SPMD, no collectives):
  - Shard NODES by src range: core c owns src in [c*NPC, (c+1)*NPC).
    Each core receives all edges whose src is in its range (all 5 types)
    and produces the final output rows for its range; host concatenates.
  - On device per core:
      Phase 1: P[t, n_local] = x[n] @ A_t + b_t          (PE matmuls)
      Phase 2: stream edges in windows:
        dma_gather   Pg = P[t, src_rel]          (512B rows)
        dma_gather   xd = x[dst]                 (512B rows)
        PE: transpose xd chunks; psum = xdT.T @ B_t; += Pg (fp32r identity mm)
        ACT: proj = relu(scale_e * psum),  scale_e = w_t / max(deg_t(src),1)
        dma_scatter_add out[src_rel] += proj
  - Scatter windows are built host-side with round-robin dealing so all
    scatter indices inside one window are unique (no DMA RMW races);
    successive windows are serialized by Tile's WAW dependency.
"""

import sys
import numpy as np

for _p in ("/opt/trn_rl_repo", "/opt/pypackages"):
    if _p not in sys.path:
        sys.path.insert(0, _p)

from concourse import bacc, mybir  # noqa: E402
import concourse.tile as tile  # noqa: E402
from concourse.masks import make_identity  # noqa: E402

F32 = mybir.dt.float32
F32R = mybir.dt.float32r
I16 = mybir.dt.int16


def full_cfg():
    return dict(
        N=50000, C=128, T=5, E=200000, NCORES=8,
        NPC=6250,          # nodes per core (src shard)
        PROWS=6272,        # padded rows per type in P table & output (49*128)
        HALF=25000,        # x row split so gather idx fits int16
        WINDOW=1024,       # tokens per scatter window (8 chunks)
        GROUP_WINS=1,      # windows per gather call (num_idxs<=1024 HW limit)
    )


def small_cfg():
    return dict(
        N=2048, C=128, T=5, E=4096, NCORES=8,
        NPC=256, PROWS=384, HALF=1024, WINDOW=256, GROUP_WINS=2,
    )


def medium_cfg():
    return dict(
        N=8192, C=128, T=5, E=65536, NCORES=8,
        NPC=1024, PROWS=1152, HALF=4096, WINDOW=1024, GROUP_WINS=1,
    )


def _softmax(v):
    v = np.asarray(v, np.float64)
    e = np.exp(v - v.max())
    return (e / e.sum()).astype(np.float32)


def build_schedule(cfg, edge_index, edge_attention):
    """Host-side: shard + bucket + deal edges into fixed-size windows.

    Returns (groups, n_tok, per_core):
      groups:   list of (t, h, n_win, tok_off); every window = WINDOW tokens
      n_tok:    tokens per core (uniform)
      per_core: dicts with pg_idx/xd_idx/sc_idx [128, n_tok//16] int16 and
                s_scale [128, n_tok//128] float32
    """
    N, T = cfg["N"], cfg["T"]
    NC, NPC, PROWS, HALF, W = (
        cfg["NCORES"], cfg["NPC"], cfg["PROWS"], cfg["HALF"], cfg["WINDOW"])
    src = np.asarray(edge_index[:, 0, :], np.int64)  # [T,E]
    dst = np.asarray(edge_index[:, 1, :], np.int64)
    w_soft = _softmax(edge_attention)

    deg = np.stack([np.bincount(src[t], minlength=N) for t in range(T)])
    scale_tbl = w_soft[:, None] / np.maximum(deg, 1).astype(np.float32)

    core_of = src // NPC
    half_of = (dst >= HALF).astype(np.int64)

    # window counts per (t,h): uniform across cores, >= max src multiplicity
    n_win = np.zeros((T, 2), np.int64)
    buckets = {}  # (c,t,h) -> edge ids sorted by src
    for t in range(T):
        for h in range(2):
            max_cnt, max_mult = 0, 0
            for c in range(NC):
                ids = np.nonzero((core_of[t] == c) & (half_of[t] == h))[0]
                if ids.size:
                    ids = ids[np.argsort(src[t][ids], kind="stable")]
                    _, cnts = np.unique(src[t][ids], return_counts=True)
                    max_mult = max(max_mult, int(cnts.max()))
                max_cnt = max(max_cnt, ids.size)
                buckets[(c, t, h)] = ids
            n_win[t, h] = max(1, -(-max_cnt // W), max_mult)

    GW = cfg["GROUP_WINS"]
    per_bucket, tok_off, win_off = {}, 0, {}
    for t in range(T):
        for h in range(2):
            win_off[(t, h)] = tok_off
            gl = []
            left = int(n_win[t, h])
            while left > 0:
                g = min(GW, left)
                gl.append((t, h, g, tok_off))
                tok_off += g * W
                left -= g
            per_bucket[(t, h)] = gl
    # round-robin across buckets so the pool engine always has an
    # independent window to prefetch while one window's scatter waits
    groups = []
    lists = [list(v) for v in per_bucket.values()]
    while any(lists):
        for gl in lists:
            if gl:
                groups.append(gl.pop(0))
    n_tok = tok_off
    assert n_tok % 128 == 0
    assert T * PROWS < 32768 and PROWS < 32768 and (N - HALF) < 32768

    per_core = []
    for c in range(NC):
        base = c * NPC
        pg = np.zeros(n_tok, np.int64)
        xd = np.zeros(n_tok, np.int64)
        sc = np.full(n_tok, NPC, np.int64)   # dummies -> trash row
        sv = np.zeros(n_tok, np.float32)
        for t in range(T):
            pg_base = t * PROWS
            for h in range(2):
                ids = buckets[(c, t, h)]
                nw = int(n_win[t, h])
                off0 = win_off[(t, h)]
                pg[off0:off0 + nw * W] = pg_base + NPC  # default: defined row
                for wdx in range(nw):
                    wids = ids[wdx::nw]
                    assert wids.size <= W, (c, t, h, wdx, wids.size)
                    o = off0 + wdx * W
                    sl = slice(o, o + wids.size)
                    s_rel = src[t][wids] - base
                    pg[sl] = pg_base + s_rel
                    xd[sl] = dst[t][wids] - HALF * h
                    sc[sl] = s_rel
                    sv[sl] = scale_tbl[t][src[t][wids]]

        def wrap16(vals):
            # token j -> [j % 16, j // 16]; the 16-partition block is
            # replicated across all 8 GPSIMD cores (partitions 16..127)
            blk = vals.reshape(n_tok // 16, 16).T
            return np.ascontiguousarray(np.tile(blk, (8, 1)).astype(np.int16))

        per_core.append(dict(
            pg_idx=wrap16(pg), xd_idx=wrap16(xd), sc_idx=wrap16(sc),
            s_scale=np.ascontiguousarray(
                sv.reshape(n_tok // 128, 128).T).astype(np.float32),
        ))
    return groups, n_tok, per_core


def build_bass(cfg, groups, n_tok):
    """Emit the Bass/Tile program (identical for all cores)."""
    C, T = cfg["C"], cfg["T"]
    NPC, PROWS, HALF, W = cfg["NPC"], cfg["PROWS"], cfg["HALF"], cfg["WINDOW"]
    XPAD = cfg["N"] - NPC + PROWS          # padded x rows
    PT = PROWS // 128                      # P tiles per type
    CW = W // 128                          # chunks per window
    QB = min(512, W)                       # tokens per PE/ACT sub-batch
    KQ = QB // 128                         # chunks per sub-batch
    n_chunks = n_tok // 128

    nc = bacc.Bacc("TRN2", target_bir_lowering=False, debug=False,
                   num_devices=cfg["NCORES"])

    x_d = nc.dram_tensor("x", [XPAD, C], F32, kind="ExternalInput").ap()
    xl_d = nc.dram_tensor("xloc", [PROWS, C], F32, kind="ExternalInput").ap()
    w_d = nc.dram_tensor("W", [T, 2 * C, C], F32, kind="ExternalInput").ap()
    b_d = nc.dram_tensor("b", [T, C], F32, kind="ExternalInput").ap()
    pgi_d = nc.dram_tensor("pg_idx", [128, n_tok // 16], I16,
                           kind="ExternalInput").ap()
    xdi_d = nc.dram_tensor("xd_idx", [128, n_tok // 16], I16,
                           kind="ExternalInput").ap()
    sci_d = nc.dram_tensor("sc_idx", [128, n_tok // 16], I16,
                           kind="ExternalInput").ap()
    s_d = nc.dram_tensor("s_scale", [128, n_chunks], F32,
                         kind="ExternalInput").ap()
    p_d = nc.dram_tensor("P", [T * PROWS, C], F32, kind="Internal").ap()
    out_d = nc.dram_tensor("out", [PROWS, C], F32, kind="ExternalOutput").ap()

    with tile.TileContext(nc) as tc:
        with (
            tc.tile_pool(name="const", bufs=1) as cpool,
            tc.tile_pool(name="xin", bufs=2) as xpool,
            tc.tile_pool(name="pout", bufs=2) as ppool,
            tc.tile_pool(name="grp", bufs=3) as gpool,
            tc.tile_pool(name="xdt", bufs=4) as tpool,
            tc.tile_pool(name="ps_pre", bufs=1, space="PSUM") as pre_ps,
            tc.tile_pool(name="ps_tr", bufs=3, space="PSUM") as tr_ps,
            tc.tile_pool(name="ps_mm", bufs=3, space="PSUM") as mm_ps,
        ):
            ident = cpool.tile([128, 128], F32)
            make_identity(nc, ident[:])
            ones = cpool.tile([1, 128], F32)
            nc.gpsimd.memset(ones[:], 1.0)
            zero = cpool.tile([128, 128], F32)
            nc.gpsimd.memset(zero[:], 0.0)

            ab = cpool.tile([128, 2 * T * 128], F32)  # [A_0|B_0|A_1|B_1|...]
            for t in range(T):
                nc.sync.dma_start(ab[:, (2 * t) * 128:(2 * t + 1) * 128],
                                  w_d[t, 0:128, :])
                nc.sync.dma_start(ab[:, (2 * t + 1) * 128:(2 * t + 2) * 128],
                                  w_d[t, 128:256, :])
            bb = cpool.tile([1, T * 128], F32)
            for t in range(T):
                nc.sync.dma_start(bb[0:1, t * 128:(t + 1) * 128],
                                  b_d[t:t + 1, :])

            pgi = cpool.tile([128, n_tok // 16], I16)
            nc.sync.dma_start(pgi[:], pgi_d[:])
            xdi = cpool.tile([128, n_tok // 16], I16)
            nc.sync.dma_start(xdi[:], xdi_d[:])
            sci = cpool.tile([128, n_tok // 16], I16)
            nc.sync.dma_start(sci[:], sci_d[:])
            ssc = cpool.tile([128, n_chunks], F32)
            nc.sync.dma_start(ssc[:], s_d[:])

            # zero the output accumulator
            out_r = out_d.rearrange("(n p) f -> p n f", p=128)
            for zk in range(PT):
                nc.sync.dma_start(out_r[:, zk:zk + 1, :],
                                  zero[:].rearrange("p (n f) -> p n f", f=128))

            # ---- Phase 1: P[t] = xloc @ A_t + ones x b_t ----
            for k in range(PT):
                xt_sb = xpool.tile([128, 128], F32, tag="xt")
                nc.sync.dma_start(xt_sb[:], xl_d[k * 128:(k + 1) * 128, :])
                ps_t = pre_ps.tile([128, 128], F32, tag="pxt")
                nc.tensor.transpose(ps_t[:], xt_sb[:], ident[:])
                xT = xpool.tile([128, 128], F32, tag="xT")
                nc.vector.tensor_copy(xT[:], ps_t[:])
                for t in range(T):
                    ps_p = pre_ps.tile([128, 128], F32, tag="pp")
                    nc.tensor.matmul(ps_p[:], xT[:],
                                     ab[:, (2 * t) * 128:(2 * t + 1) * 128],
                                     start=True, stop=False,
                                     skip_group_check=True)
                    nc.tensor.matmul(ps_p[:], ones[:],
                                     bb[0:1, t * 128:(t + 1) * 128],
                                     start=False, stop=True,
                                     skip_group_check=True)
                    p_sb = ppool.tile([128, 128], F32)
                    nc.vector.tensor_copy(p_sb[:], ps_p[:])
                    nc.sync.dma_start(
                        p_d[t * PROWS + k * 128:t * PROWS + (k + 1) * 128, :],
                        p_sb[:])

            # ---- Phase 2: edge stream ----
            relu = mybir.ActivationFunctionType.Relu
            for (t, h, nw, off) in groups:
                ntk = nw * W
                ch0 = off // 128
                pg = gpool.tile([128, ntk], F32, tag="pg")
                nc.gpsimd.dma_gather(
                    pg[:].rearrange("p (n e) -> p n e", e=128),
                    p_d[:, :], pgi[:, off // 16:(off + ntk) // 16],
                    ntk, ntk, 128)
                xg = gpool.tile([128, ntk], F32, tag="xg")
                reg = x_d[0:HALF, :] if h == 0 else x_d[HALF:XPAD, :]
                nc.gpsimd.dma_gather(
                    xg[:].rearrange("p (n e) -> p n e", e=128),
                    reg, xdi[:, off // 16:(off + ntk) // 16],
                    ntk, ntk, 128)
                proj = gpool.tile([128, ntk], F32, tag="proj")
                for wdx in range(nw):
                    for q in range(W // QB):
                        qb = wdx * W + q * QB  # token offset within group
                        ps_t = tr_ps.tile([128, QB], F32, tag="tr")
                        for k in range(KQ):
                            nc.tensor.transpose(
                                ps_t[:, k * 128:(k + 1) * 128],
                                xg[:, qb + k * 128:qb + (k + 1) * 128],
                                ident[:])
                        xdT = tpool.tile([128, QB], F32)
                        nc.vector.tensor_copy(xdT[:], ps_t[:])
                        ps_m = mm_ps.tile([128, QB], F32, tag="mm")
                        for k in range(KQ):
                            nc.tensor.matmul(
                                ps_m[:, k * 128:(k + 1) * 128],
                                xdT[:, k * 128:(k + 1) * 128],
                                ab[:, (2 * t + 1) * 128:(2 * t + 2) * 128],
                                start=True, stop=True, skip_group_check=True)
                        sm = tpool.tile([128, QB], F32, tag="sum")
                        nc.vector.tensor_add(sm[:], ps_m[:],
                                             pg[:, qb:qb + QB])
                        for k in range(KQ):
                            gc = ch0 + (qb + k * 128) // 128
                            nc.scalar.activation(
                                proj[:, qb + k * 128:qb + (k + 1) * 128],
                                sm[:, k * 128:(k + 1) * 128],
                                relu, scale=ssc[:, gc:gc + 1])
                    so = off + wdx * W
                    nc.gpsimd.dma_scatter_add(
                        out_d[:, :],
                        proj[:].rearrange("p (n e) -> p n e", e=128)[
                            :, wdx * CW:(wdx + 1) * CW, :],
                        sci[:, so // 16:(so + W) // 16], W, W, 128)

    nc.compile()
    return nc


def _in_maps(cfg, x, W, b, per_core):
    N, NPC, PROWS = cfg["N"], cfg["NPC"], cfg["PROWS"]
    XPAD = N - NPC + PROWS
    x = np.asarray(x, np.float32)
    x_pad = np.zeros((XPAD, cfg["C"]), np.float32)
    x_pad[:N] = x
    maps = []
    for c in range(cfg["NCORES"]):
        base = c * NPC
        maps.append(dict(
            x=x_pad,
            xloc=np.ascontiguousarray(x_pad[base:base + PROWS]),
            W=np.asarray(W, np.float32),
            b=np.asarray(b, np.float32),
            **per_core[c],
        ))
    return maps


def run(cfg, inputs, backend="hw", trace=False):
    """Build schedule + program, run on all cores, return [N, C] output."""
    groups, n_tok, per_core = build_schedule(
        cfg, np.asarray(inputs["edge_index"]),
        np.asarray(inputs["edge_attention"]))
    nc = build_bass(cfg, groups, n_tok)
    maps = _in_maps(cfg, inputs["x"], inputs["W"], inputs["b"], per_core)
    NPC = cfg["NPC"]

    if backend == "sim":
        import os
        from concourse.bass_interp import CoreSim
        outs = []
        for c in range(int(os.environ.get("SIM_CORES", cfg["NCORES"]))):
            sim = CoreSim(nc)
            for k, v in maps[c].items():
                sim.tensor(k)[:] = v
            if nc.partition_id_tensor is not None:
                sim.tensor(nc.partition_id_tensor.name)[:] = np.array(
                    [[c]], dtype=np.uint32)
            sim.simulate(check_with_hw=False)
            outs.append(np.array(sim.tensor("out"))[:NPC])
        return np.concatenate(outs, 0), None

    if trace:
        _install_ntff_hook()
    from concourse import bass_utils
    if trace:
        bass_utils.upload_artifacts = lambda tmpdir: "local://" + str(tmpdir)
    try:
        res = bass_utils.run_bass_kernel_spmd(
            nc, maps, core_ids=list(range(cfg["NCORES"])), trace=trace)
    except Exception:
        if not trace:
            raise
        import traceback
        traceback.print_exc()
        print("trace run failed; retrying without trace", file=sys.stderr)
        res = bass_utils.run_bass_kernel_spmd(
            nc, maps, core_ids=list(range(cfg["NCORES"])), trace=False)
    out = np.concatenate([res.results[c]["out"][:NPC]
                          for c in range(cfg["NCORES"])], 0)
    return out, res.exec_time_ns


def _install_ntff_hook():
    """Provide antenv.axon_hooks (absent from this image) and register the
    ctypes NTFF profiling hook from the axon boot helper."""
    import types
    if "antenv.axon_hooks" in sys.modules:
        return
    mod = types.ModuleType("antenv.axon_hooks")
    state = {"hook": None}
    mod.set_axon_ntff_profile_hook = lambda h: state.__setitem__("hook", h)
    mod.get_axon_ntff_profile_hook = lambda: state["hook"]
    import antenv
    sys.modules["antenv.axon_hooks"] = mod
    antenv.axon_hooks = mod
    if "/root/.axon_site" not in sys.path:
        sys.path.append("/root/.axon_site")
    try:
        from trn_agent_boot.trn_boot import _ntff_profile_via_ctypes
        mod.set_axon_ntff_profile_hook(
            _ntff_profile_via_ctypes("/opt/axon/libaxon_pjrt.so"))
    except Exception as e:  # hook stays None -> bass_utils skips tracing
        print("ntff hook unavailable:", e, file=sys.stderr)


def kernel(**inputs):
    out, _ = run(full_cfg(), inputs, backend="hw")
    return out.astype(np.float32)


# revision 24
# speedup vs baseline: 1.0424x; 1.0376x over previous
"""Trainium2 Bass kernel: NeuralLogicMachine LogicLayer message passing.

out[n] = sum_t w_t * mean_{e: src_e=n} relu(x[src_e] @ A_t + x[dst_e] @ B_t + b_t)

Strategy (8 NeuronCores, SPMD, no collectives):
  - Shard NODES by src range: core c owns src in [c*NPC, (c+1)*NPC).
    Each core receives all edges whose src is in its range (all 5 types)
    and produces the final output rows for its range; host concatenates.
  - On device per core:
      Phase 1: P[t, n_local] = x[n] @ A_t + b_t          (PE matmuls)
      Phase 2: stream edges in windows:
        dma_gather   Pg = P[t, src_rel]          (512B rows)
        dma_gather   xd = x[dst]                 (512B rows)
        PE: transpose xd chunks; psum = xdT.T @ B_t; DVE: sum = psum + Pg
        ACT: proj = relu(scale_e * sum),   scale_e = w_t / max(deg_t(src),1)
        dma_scatter_add out[src_rel] += proj
  - Scatter windows are built host-side with round-robin dealing so all
    scatter indices inside one window are unique (no DMA RMW races);
    successive windows are serialized by Tile's WAW dependency.
"""

import sys
import numpy as np

for _p in ("/opt/trn_rl_repo", "/opt/pypackages"):
    if _p not in sys.path:
        sys.path.insert(0, _p)

from concourse import bacc, mybir  # noqa: E402
import concourse.tile as tile  # noqa: E402
from concourse.masks import make_identity  # noqa: E402

F32 = mybir.dt.float32
F32R = mybir.dt.float32r
I16 = mybir.dt.int16


def full_cfg():
    return dict(
        N=50000, C=128, T=5, E=200000, NCORES=8,
        NPC=6250,          # nodes per core (src shard)
        PROWS=6272,        # padded rows per type in P table & output (49*128)
        HALF=25000,        # x row split so gather idx fits int16
        WINDOW=1024,       # tokens per scatter window (8 chunks)
        GROUP_WINS=1,      # windows per gather call (num_idxs<=1024 HW limit)
    )


def small_cfg():
    return dict(
        N=2048, C=128, T=5, E=4096, NCORES=8,
        NPC=256, PROWS=384, HALF=1024, WINDOW=256, GROUP_WINS=2,
    )


def medium_cfg():
    return dict(
        N=8192, C=128, T=5, E=65536, NCORES=8,
        NPC=1024, PROWS=1152, HALF=4096, WINDOW=1024, GROUP_WINS=1,
    )


def _softmax(v):
    v = np.asarray(v, np.float64)
    e = np.exp(v - v.max())
    return (e / e.sum()).astype(np.float32)


def build_schedule(cfg, edge_index, edge_attention):
    """Host-side: shard + bucket + deal edges into fixed-size windows.

    Returns (groups, n_tok, per_core):
      groups:   list of (t, h, n_win, tok_off); every window = WINDOW tokens
      n_tok:    tokens per core (uniform)
      per_core: dicts with pg_idx/xd_idx/sc_idx [128, n_tok//16] int16 and
                s_scale [128, n_tok//128] float32
    """
    N, T = cfg["N"], cfg["T"]
    NC, NPC, PROWS, HALF, W = (
        cfg["NCORES"], cfg["NPC"], cfg["PROWS"], cfg["HALF"], cfg["WINDOW"])
    src = np.asarray(edge_index[:, 0, :], np.int64)  # [T,E]
    dst = np.asarray(edge_index[:, 1, :], np.int64)
    w_soft = _softmax(edge_attention)

    deg = np.stack([np.bincount(src[t], minlength=N) for t in range(T)])
    scale_tbl = w_soft[:, None] / np.maximum(deg, 1).astype(np.float32)

    core_of = src // NPC
    half_of = (dst >= HALF).astype(np.int64)

    # window counts per (t,h): uniform across cores, >= max src multiplicity
    n_win = np.zeros((T, 2), np.int64)
    buckets = {}  # (c,t,h) -> edge ids sorted by src
    for t in range(T):
        for h in range(2):
            max_cnt, max_mult = 0, 0
            for c in range(NC):
                ids = np.nonzero((core_of[t] == c) & (half_of[t] == h))[0]
                if ids.size:
                    ids = ids[np.argsort(src[t][ids], kind="stable")]
                    _, cnts = np.unique(src[t][ids], return_counts=True)
                    max_mult = max(max_mult, int(cnts.max()))
                max_cnt = max(max_cnt, ids.size)
                buckets[(c, t, h)] = ids
            n_win[t, h] = max(1, -(-max_cnt // W), max_mult)

    GW = cfg["GROUP_WINS"]
    per_bucket, tok_off, win_off = {}, 0, {}
    for t in range(T):
        for h in range(2):
            win_off[(t, h)] = tok_off
            gl = []
            left = int(n_win[t, h])
            while left > 0:
                g = min(GW, left)
                gl.append((t, h, g, tok_off))
                tok_off += g * W
                left -= g
            per_bucket[(t, h)] = gl
    # round-robin across buckets so the pool engine always has an
    # independent window to prefetch while one window's scatter waits
    groups = []
    lists = [list(v) for v in per_bucket.values()]
    while any(lists):
        for gl in lists:
            if gl:
                groups.append(gl.pop(0))
    n_tok = tok_off
    assert n_tok % 128 == 0
    assert T * PROWS < 32768 and PROWS < 32768 and (N - HALF) < 32768

    per_core = []
    for c in range(NC):
        base = c * NPC
        pg = np.zeros(n_tok, np.int64)
        xd = np.zeros(n_tok, np.int64)
        sc = np.full(n_tok, NPC, np.int64)   # dummies -> trash row
        sv = np.zeros(n_tok, np.float32)
        for t in range(T):
            pg_base = t * PROWS
            for h in range(2):
                ids = buckets[(c, t, h)]
                nw = int(n_win[t, h])
                off0 = win_off[(t, h)]
                pg[off0:off0 + nw * W] = pg_base + NPC  # default: defined row
                for wdx in range(nw):
                    wids = ids[wdx::nw]
                    assert wids.size <= W, (c, t, h, wdx, wids.size)
                    o = off0 + wdx * W
                    sl = slice(o, o + wids.size)
                    s_rel = src[t][wids] - base
                    pg[sl] = pg_base + s_rel
                    xd[sl] = dst[t][wids] - HALF * h
                    sc[sl] = s_rel
                    sv[sl] = scale_tbl[t][src[t][wids]]

        def wrap16(vals):
            # token j -> [j % 16, j // 16]; the 16-partition block is
            # replicated across all 8 GPSIMD cores (partitions 16..127)
            blk = vals.reshape(n_tok // 16, 16).T
            return np.ascontiguousarray(np.tile(blk, (8, 1)).astype(np.int16))

        per_core.append(dict(
            pg_idx=wrap16(pg), xd_idx=wrap16(xd), sc_idx=wrap16(sc),
            s_scale=np.ascontiguousarray(
                sv.reshape(n_tok // 128, 128).T).astype(np.float32),
        ))
    return groups, n_tok, per_core


def build_bass(cfg, groups, n_tok):
    """Emit the Bass/Tile program (identical for all cores)."""
    C, T = cfg["C"], cfg["T"]
    NPC, PROWS, HALF, W = cfg["NPC"], cfg["PROWS"], cfg["HALF"], cfg["WINDOW"]
    XPAD = cfg["N"] - NPC + PROWS          # padded x rows
    PT = PROWS // 128                      # P tiles per type
    CW = W // 128                          # chunks per window
    QB = min(512, W)                       # tokens per PE/ACT sub-batch
    KQ = QB // 128                         # chunks per sub-batch
    n_chunks = n_tok // 128

    nc = bacc.Bacc("TRN2", target_bir_lowering=False, debug=False,
                   num_devices=cfg["NCORES"])

    x_d = nc.dram_tensor("x", [XPAD, C], F32, kind="ExternalInput").ap()
    xl_d = nc.dram_tensor("xloc", [PROWS, C], F32, kind="ExternalInput").ap()
    w_d = nc.dram_tensor("W", [T, 2 * C, C], F32, kind="ExternalInput").ap()
    b_d = nc.dram_tensor("b", [T, C], F32, kind="ExternalInput").ap()
    pgi_d = nc.dram_tensor("pg_idx", [128, n_tok // 16], I16,
                           kind="ExternalInput").ap()
    xdi_d = nc.dram_tensor("xd_idx", [128, n_tok // 16], I16,
                           kind="ExternalInput").ap()
    sci_d = nc.dram_tensor("sc_idx", [128, n_tok // 16], I16,
                           kind="ExternalInput").ap()
    s_d = nc.dram_tensor("s_scale", [128, n_chunks], F32,
                         kind="ExternalInput").ap()
    p_d = nc.dram_tensor("P", [T * PROWS, C], F32, kind="Internal").ap()
    out_d = nc.dram_tensor("out", [PROWS, C], F32, kind="ExternalOutput").ap()

    with tile.TileContext(nc) as tc:
        with (
            tc.tile_pool(name="const", bufs=1) as cpool,
            tc.tile_pool(name="xin", bufs=2) as xpool,
            tc.tile_pool(name="pout", bufs=2) as ppool,
            tc.tile_pool(name="grp", bufs=3) as gpool,
            tc.tile_pool(name="xdt", bufs=4) as tpool,
            tc.tile_pool(name="ps_pre", bufs=1, space="PSUM") as pre_ps,
            tc.tile_pool(name="ps_tr", bufs=3, space="PSUM") as tr_ps,
            tc.tile_pool(name="ps_mm", bufs=3, space="PSUM") as mm_ps,
        ):
            ident = cpool.tile([128, 128], F32)
            make_identity(nc, ident[:])
            ones = cpool.tile([1, 128], F32)
            nc.gpsimd.memset(ones[:], 1.0)
            zero = cpool.tile([128, 128], F32)
            nc.gpsimd.memset(zero[:], 0.0)

            ab = cpool.tile([128, 2 * T * 128], F32)  # [A_0|B_0|A_1|B_1|...]
            for t in range(T):
                nc.sync.dma_start(ab[:, (2 * t) * 128:(2 * t + 1) * 128],
                                  w_d[t, 0:128, :])
                nc.sync.dma_start(ab[:, (2 * t + 1) * 128:(2 * t + 2) * 128],
                                  w_d[t, 128:256, :])
            bb = cpool.tile([1, T * 128], F32)
            for t in range(T):
                nc.sync.dma_start(bb[0:1, t * 128:(t + 1) * 128],
                                  b_d[t:t + 1, :])

            pgi = cpool.tile([128, n_tok // 16], I16)
            nc.sync.dma_start(pgi[:], pgi_d[:])
            xdi = cpool.tile([128, n_tok // 16], I16)
            nc.sync.dma_start(xdi[:], xdi_d[:])
            sci = cpool.tile([128, n_tok // 16], I16)
            nc.sync.dma_start(sci[:], sci_d[:])
            ssc = cpool.tile([128, n_chunks], F32)
            nc.sync.dma_start(ssc[:], s_d[:])

            # zero the output accumulator
            out_r = out_d.rearrange("(n p) f -> p n f", p=128)
            for zk in range(PT):
                nc.sync.dma_start(out_r[:, zk:zk + 1, :],
                                  zero[:].rearrange("p (n f) -> p n f", f=128))

            # ---- Phase 1: P[t] = xloc @ A_t + ones x b_t ----
            for k in range(PT):
                xt_sb = xpool.tile([128, 128], F32, tag="xt")
                nc.sync.dma_start(xt_sb[:], xl_d[k * 128:(k + 1) * 128, :])
                ps_t = pre_ps.tile([128, 128], F32, tag="pxt")
                nc.tensor.transpose(ps_t[:], xt_sb[:], ident[:])
                xT = xpool.tile([128, 128], F32, tag="xT")
                nc.vector.tensor_copy(xT[:], ps_t[:])
                for t in range(T):
                    ps_p = pre_ps.tile([128, 128], F32, tag="pp")
                    nc.tensor.matmul(ps_p[:], xT[:],
                                     ab[:, (2 * t) * 128:(2 * t + 1) * 128],
                                     start=True, stop=False,
                                     skip_group_check=True)
                    nc.tensor.matmul(ps_p[:], ones[:],
                                     bb[0:1, t * 128:(t + 1) * 128],
                                     start=False, stop=True,
                                     skip_group_check=True)
                    p_sb = ppool.tile([128, 128], F32)
                    nc.vector.tensor_copy(p_sb[:], ps_p[:])
                    nc.sync.dma_start(
                        p_d[t * PROWS + k * 128:t * PROWS + (k + 1) * 128, :],
                        p_sb[:])

            # ---- Phase 2: edge stream ----
            relu = mybir.ActivationFunctionType.Relu
            for (t, h, nw, off) in groups:
                ntk = nw * W
                ch0 = off // 128
                pg = gpool.tile([128, ntk], F32, tag="pg")
                nc.gpsimd.dma_gather(
                    pg[:].rearrange("p (n e) -> p n e", e=128),
                    p_d[:, :], pgi[:, off // 16:(off + ntk) // 16],
                    ntk, ntk, 128)
                xg = gpool.tile([128, ntk], F32, tag="xg")
                reg = x_d[0:HALF, :] if h == 0 else x_d[HALF:XPAD, :]
                nc.gpsimd.dma_gather(
                    xg[:].rearrange("p (n e) -> p n e", e=128),
                    reg, xdi[:, off // 16:(off + ntk) // 16],
                    ntk, ntk, 128)
                proj = gpool.tile([128, ntk], F32, tag="proj")
                for wdx in range(nw):
                    for q in range(W // QB):
                        qb = wdx * W + q * QB  # token offset within group
                        ps_t = tr_ps.tile([128, QB], F32, tag="tr")
                        for k in range(KQ):
                            nc.tensor.transpose(
                                ps_t[:, k * 128:(k + 1) * 128],
                                xg[:, qb + k * 128:qb + (k + 1) * 128],
                                ident[:])
                        xdT = tpool.tile([128, QB], F32)
                        nc.vector.tensor_copy(xdT[:], ps_t[:])
                        ps_m = mm_ps.tile([128, QB], F32, tag="mm")
                        for k in range(KQ):
                            nc.tensor.matmul(
                                ps_m[:, k * 128:(k + 1) * 128],
                                xdT[:, k * 128:(k + 1) * 128],
                                ab[:, (2 * t + 1) * 128:(2 * t + 2) * 128],
                                start=True, stop=True, skip_group_check=True)
                        sm = tpool.tile([128, QB], F32, tag="sum")
                        nc.vector.tensor_add(sm[:], ps_m[:],
                                             pg[:, qb:qb + QB])
                        for k in range(KQ):
                            gc = ch0 + (qb + k * 128) // 128
                            nc.scalar.activation(
                                proj[:, qb + k * 128:qb + (k + 1) * 128],
                                sm[:, k * 128:(k + 1) * 128],
                                relu, scale=ssc[:, gc:gc + 1])
                    so = off + wdx * W
                    nc.gpsimd.dma_scatter_add(
                        out_d[:, :],
                        proj[:].rearrange("p (n e) -> p n e", e=128)[
                            :, wdx * CW:(wdx + 1) * CW, :],
                        sci[:, so // 16:(so + W) // 16], W, W, 128)

    nc.compile()
    return nc


def _in_maps(cfg, x, W, b, per_core):
    N, NPC, PROWS = cfg["N"], cfg["NPC"], cfg["PROWS"]
    XPAD = N - NPC + PROWS
    x = np.asarray(x, np.float32)
    x_pad = np.zeros((XPAD, cfg["C"]), np.float32)
    x_pad[:N] = x
    maps = []
    for c in range(cfg["NCORES"]):
        base = c * NPC
        maps.append(dict(
            x=x_pad,
            xloc=np.ascontiguousarray(x_pad[base:base + PROWS]),
            W=np.asarray(W, np.float32),
            b=np.asarray(b, np.float32),
            **per_core[c],
        ))
    return maps


def run(cfg, inputs, backend="hw", trace=False):
    """Build schedule + program, run on all cores, return [N, C] output."""
    groups, n_tok, per_core = build_schedule(
        cfg, np.asarray(inputs["edge_index"]),
        np.asarray(inputs["edge_attention"]))
    nc = build_bass(cfg, groups, n_tok)
    maps = _in_maps(cfg, inputs["x"], inputs["W"], inputs["b"], per_core)
    NPC = cfg["NPC"]

    if backend == "sim":
        import os
        from concourse.bass_interp import CoreSim
        outs = []
        for c in range(int(os.environ.get("SIM_CORES", cfg["NCORES"]))):
            sim = CoreSim(nc)
            for k, v in maps[c].items():
                sim.tensor(k)[:] = v
            if nc.partition_id_tensor is not None:
                sim.tensor(nc.partition_id_tensor.name)[:] = np.array(
                    [[c]], dtype=np.uint32)
            sim.simulate(check_with_hw=False)
            outs.append(np.array(sim.tensor("out"))[:NPC])
        return np.concatenate(outs, 0), None

    if trace:
        _install_ntff_hook()
    from concourse import bass_utils
    if trace:
        bass_utils.upload_artifacts = lambda tmpdir: "local://" + str(tmpdir)
    try:
        res = bass_utils.run_bass_kernel_spmd(
            nc, maps, core_ids=list(range(cfg["NCORES"])), trace=trace)
    except Exception:
        if not trace:
            raise
        import traceback
        traceback.print_exc()
        print("trace run failed; retrying without trace", file=sys.stderr)
        res = bass_utils.run_bass_kernel_spmd(
            nc, maps, core_ids=list(range(cfg["NCORES"])), trace=False)
    out = np.concatenate([res.results[c]["out"][:NPC]
                          for c in range(cfg["NCORES"])], 0)
    return out, res.exec_time_ns


def _install_ntff_hook():
    """Provide antenv.axon_hooks (absent from this image) and register the
    ctypes NTFF profiling hook from the axon boot helper."""
    import types
    if "antenv.axon_hooks" in sys.modules:
        return
    mod = types.ModuleType("antenv.axon_hooks")
    state = {"hook": None}
    mod.set_axon_ntff_profile_hook = lambda h: state.__setitem__("hook", h)
    mod.get_axon_ntff_profile_hook = lambda: state["hook"]
    import antenv
    sys.modules["antenv.axon_hooks"] = mod
    antenv.axon_hooks = mod
    if "/root/.axon_site" not in sys.path:
        sys.path.append("/root/.axon_site")
    try:
        from trn_agent_boot.trn_boot import _ntff_profile_via_ctypes
        mod.set_axon_ntff_profile_hook(
            _ntff_profile_via_ctypes("/opt/axon/libaxon_pjrt.so"))
    except Exception as e:  # hook stays None -> bass_utils skips tracing
        print("ntff hook unavailable:", e, file=sys.stderr)


def kernel(**inputs):
    out, _ = run(full_cfg(), inputs, backend="hw")
    return out.astype(np.float32)
